# revision 4
# baseline (speedup 1.0000x reference)
"""RNN-T Joiner kernel for Trainium2, SPMD over 8 NeuronCores.

Reference computation (per batch b):
    hf = ft[b] @ w1[:, :ENC].T            # [T, J]
    hg = gu[b] @ w1[:, ENC:].T            # [U, J]
    joint = tanh(hf[:, None, :] + hg[None, :, :])   # [T, U, J]
    out[b] = joint @ w2.T                 # [T, U, V]

Sharding: data-parallel over B — each of the 8 cores handles one batch
element, full weights replicated. No collectives.

Per-core layout strategy: keep J (the contraction dim of the big GEMM) on
SBUF partitions. hfT[j, t] and hgT[j, u] are produced by small fp32 GEMMs
from PE-transposed inputs. Then for each u, a ScalarE activation computes
jointT[j, t] = tanh(hfT + hgT[:, u]) with the per-partition bias feature,
writing fp16. The big GEMM runs in fp16 (full PE rate, half-cost
LdWeights vs fp32) producing [t-chunk, V] fp32 tiles in PSUM. PSUM
evacuation converts to fp16 and is split across the Vector (DVE) and
GpSimd (Pool) engines so neither throttles the PE. Output is DMA'd to
DRAM in fp16 (halving HBM write traffic vs fp32) and upcast to fp32 on
the host; fp16 quantization error (~1e-3 relative) is far below the
2e-2 gate.
"""

import numpy as np

import concourse.bass as bass
import concourse.mybir as mybir
import concourse.tile as tile
from concourse import bacc
from concourse.bass_utils import run_bass_kernel_spmd
from concourse.masks import make_identity

B, T, U = 8, 256, 64
ENC, PRED = 128, 256
J, V = 256, 500
N_CORES = 8
P = 128
f32 = mybir.dt.float32
f32r = mybir.dt.float32r
f16 = mybir.dt.float16


def _emit(nc, tc, ft, gu, w1, w2, out):
    JO = J // P          # 2 chunks of j
    TO = T // P          # 2 chunks of t
    with (
        tc.tile_pool(name="const", bufs=1) as const,
        tc.tile_pool(name="joint", bufs=4) as jpool,
    ):
        ident = const.tile([P, P], f32)
        make_identity(nc, ident)

        # ---- natural-layout loads ----
        w1_sb = const.tile([P, JO, ENC + PRED], f32)  # [j, jo, e|p]
        nc.sync.dma_start(w1_sb[:], w1.ap().rearrange("(jo p) e -> p jo e", p=P))
        w2_sb = const.tile([P, 4, J], f32)            # [v(125 pad 128), vo, j]
        nc.any.memzero(w2_sb[:])
        nc.sync.dma_start(
            w2_sb[:125, :, :], w2.ap().rearrange("(vo p) j -> p vo j", p=125)
        )
        ft_sb = const.tile([P, TO, ENC], f32)         # [t, to, e]
        nc.sync.dma_start(ft_sb[:], ft.ap().rearrange("(to p) e -> p to e", p=P))
        gu_sb = const.tile([P, PRED], f32)            # [u (pad 128), p]
        nc.any.memzero(gu_sb[:])
        nc.sync.dma_start(gu_sb[:U, :], gu.ap())

        # ---- PE transposes into contraction-major layouts ----
        pst_cm = tc.tile_pool(name="pst", bufs=2, space="PSUM")
        pst = pst_cm.__enter__()
        # w1T[k, kc, j]: w1T[:, kc, jo*P+jj] = w1[jo*P+jj, kc*P+k]
        w1T = const.tile([P, 3, J], f32)
        for kc in range(3):
            for jo in range(JO):
                pt = pst.tile([P, P], f32, tag="pt")
                nc.tensor.transpose(
                    pt[:], w1_sb[:, jo, kc * P : (kc + 1) * P], ident[:]
                )
                nc.vector.tensor_copy(w1T[:, kc, jo * P : (jo + 1) * P], pt[:])

        # w2T[j, jo, v] (fp16 for the big GEMM)
        w2T = const.tile([P, JO, V], f16)
        for vo in range(4):
            for jo in range(JO):
                pt = pst.tile([P, P], f32, tag="pt")
                nc.tensor.transpose(
                    pt[:], w2_sb[:, vo, jo * P : (jo + 1) * P], ident[:]
                )
                nc.vector.tensor_copy(
                    w2T[:, jo, vo * 125 : (vo + 1) * 125], pt[:, :125]
                )

        # ftT[e, t]
        ftT = const.tile([P, T], f32)
        for to in range(TO):
            pt = pst.tile([P, P], f32, tag="pt")
            nc.tensor.transpose(pt[:], ft_sb[:, to, :], ident[:])
            nc.vector.tensor_copy(ftT[:, to * P : (to + 1) * P], pt[:])

        # guT[p, pc, u]
        guT = const.tile([P, PRED // P, U], f32)
        for pc in range(PRED // P):
            pt = pst.tile([P, P], f32, tag="pt")
            nc.tensor.transpose(pt[:], gu_sb[:, pc * P : (pc + 1) * P], ident[:])
            nc.vector.tensor_copy(guT[:, pc, :], pt[:, :U])

        # ---- first GEMMs (fp32): hfT[j, t], hgT[j, u] ----
        hfT = const.tile([P, JO, T], f32)
        for jo in range(JO):
            ph = pst.tile([P, T], f32, tag="ph")
            nc.tensor.matmul(
                ph[:], w1T[:, 0, jo * P : (jo + 1) * P], ftT[:], start=True, stop=True
            )
            nc.vector.tensor_copy(hfT[:, jo, :], ph[:])

        hgT = const.tile([P, JO, U], f32)
        for jo in range(JO):
            ph = pst.tile([P, U], f32, tag="phg")
            for pc in range(PRED // P):
                nc.tensor.matmul(
                    ph[:],
                    w1T[:, 1 + pc, jo * P : (jo + 1) * P],
                    guT[:, pc, :],
                    start=(pc == 0),
                    stop=(pc == 1),
                )
            nc.vector.tensor_copy(hgT[:, jo, :], ph[:])

        pst_cm.__exit__(None, None, None)

        # ---- main loop over u, in blocks of UB for batched output DMA ----
        # Per UB=4 block: 8 broadcast-adds pre = hf + hg[u] (7 on GpSimd,
        # 1 on DVE — GpSimd can't read PSUM but is otherwise idle), then
        # ONE tanh ACT per jo over all 4 u's (1024 free elems — amortizes
        # the scalar engine's ~270ns fixed per-instruction cost that
        # dominated when each (u, jo) was its own 256-elem bias'd ACT).
        # 4 fp16 matmuls per u into a 2-bank PSUM tile, evacuated by one
        # fused 1000-elem fp32->fp16 copy (3 per block on DVE, 1 on
        # scalar).
        UB = 4
        pso_cm = tc.tile_pool(name="pso", bufs=3, space="PSUM")
        pso = pso_cm.__enter__()
        for ub in range(U // UB):
            ot = jpool.tile([P, TO, UB, V], f16, tag="ot", name="ot")
            pre = jpool.tile([P, JO, UB, T], f16, tag="pre", name="pre")
            jt = jpool.tile([P, JO, UB, T], f16, tag="joint", name="joint")
            for uo in range(UB):
                u = ub * UB + uo
                for jo in range(JO):
                    eng = nc.vector if (uo == UB - 1 and jo == 1) else nc.gpsimd
                    eng.tensor_scalar_add(
                        pre[:, jo, uo, :], hfT[:, jo, :], hgT[:, jo, u : u + 1]
                    )
            for jo in range(JO):
                nc.scalar.activation(
                    jt[:, jo, :, :],
                    pre[:, jo, :, :],
                    mybir.ActivationFunctionType.Tanh,
                )
            for uo in range(UB):
                # 512-padded second bank so both matmul outputs are
                # bank-aligned
                po = pso.tile([P, TO, 512], f32, tag="po")
                for to in range(TO):
                    for jo in range(JO):
                        nc.tensor.matmul(
                            po[:, to, 0:V],
                            jt[:, jo, uo, to * P : (to + 1) * P],
                            w2T[:, jo, :],
                            start=(jo == 0),
                            stop=(jo == JO - 1),
                        )
                if uo == UB - 1:
                    nc.scalar.copy(ot[:, :, uo, :], po[:, :, 0:V])
                else:
                    nc.vector.tensor_copy(ot[:, :, uo, :], po[:, :, 0:V])
            for to in range(TO):
                nc.sync.dma_start(
                    out.ap()[to * P : (to + 1) * P, ub * UB : (ub + 1) * UB, :],
                    ot[:, to, :, :],
                )
        pso_cm.__exit__(None, None, None)


_NC_CACHE = None


def _build():
    global _NC_CACHE
    if _NC_CACHE is not None:
        return _NC_CACHE
    nc = bacc.Bacc("TRN2", target_bir_lowering=False, debug=False)
    ft = nc.dram_tensor("ft", [T, ENC], f32, kind="ExternalInput")
    gu = nc.dram_tensor("gu", [U, PRED], f32, kind="ExternalInput")
    w1 = nc.dram_tensor("w1", [J, ENC + PRED], f32, kind="ExternalInput")
    w2 = nc.dram_tensor("w2", [V, J], f32, kind="ExternalInput")
    out = nc.dram_tensor("out", [T, U, V], f16, kind="ExternalOutput")
    with tile.TileContext(nc) as tc:
        _emit(nc, tc, ft, gu, w1, w2, out)
    nc.compile()
    _NC_CACHE = nc
    return nc


def run(ft, gu, w1, w2, trace=False):
    """Run the SPMD kernel; returns (output [B,T,U,V], BassKernelResults)."""
    nc = _build()
    w1c = np.ascontiguousarray(w1, dtype=np.float32)
    w2c = np.ascontiguousarray(w2, dtype=np.float32)
    in_maps = [
        {
            "ft": np.ascontiguousarray(ft[b], dtype=np.float32),
            "gu": np.ascontiguousarray(gu[b], dtype=np.float32),
            "w1": w1c,
            "w2": w2c,
        }
        for b in range(B)
    ]
    res = run_bass_kernel_spmd(
        nc, in_maps, core_ids=list(range(N_CORES)), trace=trace
    )
    out = np.stack(
        [res.results[c]["out"].astype(np.float32) for c in range(N_CORES)], axis=0
    )
    return out, res


def kernel(ft, gu, w1, w2):
    out, _ = run(ft, gu, w1, w2, trace=False)
    return out


# revision 5
# speedup vs baseline: 3.9354x; 3.9354x over previous
"""RNN-T Joiner kernel for Trainium2, SPMD over 8 NeuronCores.

Reference computation (per batch b):
    hf = ft[b] @ w1[:, :ENC].T            # [T, J]
    hg = gu[b] @ w1[:, ENC:].T            # [U, J]
    joint = tanh(hf[:, None, :] + hg[None, :, :])   # [T, U, J]
    out[b] = joint @ w2.T                 # [T, U, V]

Sharding: data-parallel over B — each of the 8 cores handles one batch
element, full weights replicated. No collectives.

Per-core layout strategy: keep J (the contraction dim of the big GEMM) on
SBUF partitions. hfT[j, t] and hgT[j, u] are produced by small fp32 GEMMs
from PE-transposed inputs. Then for each u, a ScalarE activation computes
jointT[j, t] = tanh(hfT + hgT[:, u]) with the per-partition bias feature,
writing fp16. The big GEMM runs in fp16 (full PE rate, half-cost
LdWeights vs fp32) producing [t-chunk, V] fp32 tiles in PSUM. PSUM
evacuation converts to fp16 and is split across the Vector (DVE) and
GpSimd (Pool) engines so neither throttles the PE. Output is DMA'd to
DRAM in fp16 (halving HBM write traffic vs fp32) and upcast to fp32 on
the host; fp16 quantization error (~1e-3 relative) is far below the
2e-2 gate.
"""

import numpy as np

import concourse.bass as bass
import concourse.mybir as mybir
import concourse.tile as tile
from concourse import bacc
from concourse.bass_utils import run_bass_kernel_spmd
from concourse.masks import make_identity

B, T, U = 8, 256, 64
ENC, PRED = 128, 256
J, V = 256, 500
N_CORES = 8
P = 128
f32 = mybir.dt.float32
f32r = mybir.dt.float32r
f16 = mybir.dt.float16


def _emit(nc, tc, ft, gu, w1, w2, out):
    JO = J // P          # 2 chunks of j
    TO = T // P          # 2 chunks of t
    with (
        tc.tile_pool(name="const", bufs=1) as const,
        tc.tile_pool(name="joint", bufs=4) as jpool,
    ):
        ident = const.tile([P, P], f32)
        make_identity(nc, ident)

        # ---- natural-layout loads ----
        w1_sb = const.tile([P, JO, ENC + PRED], f32)  # [j, jo, e|p]
        nc.sync.dma_start(w1_sb[:], w1.ap().rearrange("(jo p) e -> p jo e", p=P))
        w2_sb = const.tile([P, 4, J], f32)            # [v(125 pad 128), vo, j]
        nc.any.memzero(w2_sb[:])
        nc.sync.dma_start(
            w2_sb[:125, :, :], w2.ap().rearrange("(vo p) j -> p vo j", p=125)
        )
        ft_sb = const.tile([P, TO, ENC], f32)         # [t, to, e]
        nc.sync.dma_start(ft_sb[:], ft.ap().rearrange("(to p) e -> p to e", p=P))
        gu_sb = const.tile([P, PRED], f32)            # [u (pad 128), p]
        nc.any.memzero(gu_sb[:])
        nc.sync.dma_start(gu_sb[:U, :], gu.ap())

        # ---- PE transposes into contraction-major layouts ----
        pst_cm = tc.tile_pool(name="pst", bufs=2, space="PSUM")
        pst = pst_cm.__enter__()
        # w1T[k, kc, j]: w1T[:, kc, jo*P+jj] = w1[jo*P+jj, kc*P+k]
        w1T = const.tile([P, 3, J], f32)
        for kc in range(3):
            for jo in range(JO):
                pt = pst.tile([P, P], f32, tag="pt")
                nc.tensor.transpose(
                    pt[:], w1_sb[:, jo, kc * P : (kc + 1) * P], ident[:]
                )
                nc.vector.tensor_copy(w1T[:, kc, jo * P : (jo + 1) * P], pt[:])

        # w2T[j, jo, v] (fp16 for the big GEMM)
        w2T = const.tile([P, JO, V], f16)
        for vo in range(4):
            for jo in range(JO):
                pt = pst.tile([P, P], f32, tag="pt")
                nc.tensor.transpose(
                    pt[:], w2_sb[:, vo, jo * P : (jo + 1) * P], ident[:]
                )
                nc.vector.tensor_copy(
                    w2T[:, jo, vo * 125 : (vo + 1) * 125], pt[:, :125]
                )

        # ftT[e, t]
        ftT = const.tile([P, T], f32)
        for to in range(TO):
            pt = pst.tile([P, P], f32, tag="pt")
            nc.tensor.transpose(pt[:], ft_sb[:, to, :], ident[:])
            nc.vector.tensor_copy(ftT[:, to * P : (to + 1) * P], pt[:])

        # guT[p, pc, u]
        guT = const.tile([P, PRED // P, U], f32)
        for pc in range(PRED // P):
            pt = pst.tile([P, P], f32, tag="pt")
            nc.tensor.transpose(pt[:], gu_sb[:, pc * P : (pc + 1) * P], ident[:])
            nc.vector.tensor_copy(guT[:, pc, :], pt[:, :U])

        # ---- first GEMMs (fp32): hfT[j, t], hgT[j, u] ----
        hfT = const.tile([P, JO, T], f32)
        for jo in range(JO):
            ph = pst.tile([P, T], f32, tag="ph")
            nc.tensor.matmul(
                ph[:], w1T[:, 0, jo * P : (jo + 1) * P], ftT[:], start=True, stop=True
            )
            nc.vector.tensor_copy(hfT[:, jo, :], ph[:])

        hgT = const.tile([P, JO, U], f32)
        for jo in range(JO):
            ph = pst.tile([P, U], f32, tag="phg")
            for pc in range(PRED // P):
                nc.tensor.matmul(
                    ph[:],
                    w1T[:, 1 + pc, jo * P : (jo + 1) * P],
                    guT[:, pc, :],
                    start=(pc == 0),
                    stop=(pc == 1),
                )
            nc.vector.tensor_copy(hgT[:, jo, :], ph[:])

        pst_cm.__exit__(None, None, None)

        # ---- main loop over u, in blocks of UB for batched output DMA ----
        # Per UB=4 block: one DVE broadcast-add per jo computes
        # pre[j, u, t] = hf[j, t] + hg[j, u] via 0-stride APs (memory
        # streamed — per-partition register operands are slow on DVE),
        # then ONE tanh ACT per jo over all 4 u's (1024 free elems —
        # amortizes the scalar engine's ~270ns fixed per-instruction cost
        # that dominated when each (u, jo) was its own 256-elem bias'd
        # ACT). 4 fp16 matmuls per u into a 2-bank PSUM tile, evacuated
        # by one fused 1000-elem fp32->fp16 copy, split ~1.5:2.5 between
        # the scalar engine and DVE.
        UB = 4
        pso_cm = tc.tile_pool(name="pso", bufs=4, space="PSUM")
        pso = pso_cm.__enter__()
        for ub in range(U // UB):
            ot = jpool.tile([P, TO, UB, V], f16, tag="ot", name="ot")
            pre = jpool.tile([P, JO, UB, T], f16, tag="pre", name="pre")
            jt = jpool.tile([P, JO, UB, T], f16, tag="joint", name="joint")
            for jo in range(JO):
                nc.vector.tensor_add(
                    pre[:, jo, :, :],
                    hfT[:, jo, :].unsqueeze(1).broadcast_to([P, UB, T]),
                    hgT[:, jo, ub * UB : (ub + 1) * UB]
                    .unsqueeze(2)
                    .broadcast_to([P, UB, T]),
                )
            for jo in range(JO):
                nc.scalar.activation(
                    jt[:, jo, :, :],
                    pre[:, jo, :, :],
                    mybir.ActivationFunctionType.Tanh,
                )
            for uo in range(UB):
                # 512-padded second bank so both matmul outputs are
                # bank-aligned
                po = pso.tile([P, TO, 512], f32, tag="po")
                for to in range(TO):
                    for jo in range(JO):
                        nc.tensor.matmul(
                            po[:, to, 0:V],
                            jt[:, jo, uo, to * P : (to + 1) * P],
                            w2T[:, jo, :],
                            start=(jo == 0),
                            stop=(jo == JO - 1),
                        )
                if uo == UB - 1 or (uo == 1 and ub % 2 == 0):
                    nc.scalar.copy(ot[:, :, uo, :], po[:, :, 0:V])
                else:
                    nc.vector.tensor_copy(ot[:, :, uo, :], po[:, :, 0:V])
            for to in range(TO):
                nc.sync.dma_start(
                    out.ap()[to * P : (to + 1) * P, ub * UB : (ub + 1) * UB, :],
                    ot[:, to, :, :],
                )
        pso_cm.__exit__(None, None, None)


_NC_CACHE = None


def _build():
    global _NC_CACHE
    if _NC_CACHE is not None:
        return _NC_CACHE
    nc = bacc.Bacc("TRN2", target_bir_lowering=False, debug=False)
    ft = nc.dram_tensor("ft", [T, ENC], f32, kind="ExternalInput")
    gu = nc.dram_tensor("gu", [U, PRED], f32, kind="ExternalInput")
    w1 = nc.dram_tensor("w1", [J, ENC + PRED], f32, kind="ExternalInput")
    w2 = nc.dram_tensor("w2", [V, J], f32, kind="ExternalInput")
    out = nc.dram_tensor("out", [T, U, V], f16, kind="ExternalOutput")
    with tile.TileContext(nc) as tc:
        _emit(nc, tc, ft, gu, w1, w2, out)
    nc.compile()
    _NC_CACHE = nc
    return nc


def run(ft, gu, w1, w2, trace=False):
    """Run the SPMD kernel; returns (output [B,T,U,V], BassKernelResults)."""
    nc = _build()
    w1c = np.ascontiguousarray(w1, dtype=np.float32)
    w2c = np.ascontiguousarray(w2, dtype=np.float32)
    in_maps = [
        {
            "ft": np.ascontiguousarray(ft[b], dtype=np.float32),
            "gu": np.ascontiguousarray(gu[b], dtype=np.float32),
            "w1": w1c,
            "w2": w2c,
        }
        for b in range(B)
    ]
    res = run_bass_kernel_spmd(
        nc, in_maps, core_ids=list(range(N_CORES)), trace=trace
    )
    out = np.stack(
        [res.results[c]["out"].astype(np.float32) for c in range(N_CORES)], axis=0
    )
    return out, res


def kernel(ft, gu, w1, w2):
    out, _ = run(ft, gu, w1, w2, trace=False)
    return out


# revision 14
# speedup vs baseline: 4.4869x; 1.1401x over previous
"""RNN-T Joiner kernel for Trainium2, SPMD over 8 NeuronCores.

Reference computation (per batch b):
    hf = ft[b] @ w1[:, :ENC].T            # [T, J]
    hg = gu[b] @ w1[:, ENC:].T            # [U, J]
    joint = tanh(hf[:, None, :] + hg[None, :, :])   # [T, U, J]
    out[b] = joint @ w2.T                 # [T, U, V]

Sharding: data-parallel over B — each of the 8 cores handles one batch
element, full weights replicated. No collectives.

Per-core layout strategy: keep J (the contraction dim of the big GEMM) on
SBUF partitions. hfT[j, t] and hgT[j, u] are produced by small fp32 GEMMs
from PE-transposed inputs. Then for each u, a ScalarE activation computes
jointT[j, t] = tanh(hfT + hgT[:, u]) with the per-partition bias feature,
writing fp16. The big GEMM runs in fp16 (full PE rate, half-cost
LdWeights vs fp32) producing [t-chunk, V] fp32 tiles in PSUM. PSUM
evacuation converts to fp16 and is split across the Vector (DVE) and
GpSimd (Pool) engines so neither throttles the PE. Output is DMA'd to
DRAM in fp16 (halving HBM write traffic vs fp32) and upcast to fp32 on
the host; fp16 quantization error (~1e-3 relative) is far below the
2e-2 gate.
"""

import numpy as np

import concourse.bass as bass
import concourse.mybir as mybir
import concourse.tile as tile
from concourse import bacc
from concourse.bass_utils import run_bass_kernel_spmd
from concourse.masks import make_identity

B, T, U = 8, 256, 64
ENC, PRED = 128, 256
J, V = 256, 500
N_CORES = 8
P = 128
f32 = mybir.dt.float32
f32r = mybir.dt.float32r
f16 = mybir.dt.float16


def _emit(nc, tc, ft, gu, w1, w2, out):
    JO = J // P          # 2 chunks of j
    TO = T // P          # 2 chunks of t
    with (
        tc.tile_pool(name="const", bufs=1) as const,
        tc.tile_pool(name="joint", bufs=8) as jpool,
    ):
        ident = const.tile([P, P], f32)
        make_identity(nc, ident)

        # ---- natural-layout loads ----
        # ft/gu/w1 first: they feed the hfT/hgT critical path; the large
        # w2 load only gates the first big matmul, ~2us later.
        ft_sb = const.tile([P, TO, ENC], f32)         # [t, to, e]
        nc.sync.dma_start(ft_sb[:], ft.ap().rearrange("(to p) e -> p to e", p=P))
        gu_sb = const.tile([P, PRED], f32)            # [u (pad 128), p]
        nc.any.memzero(gu_sb[:])
        nc.sync.dma_start(gu_sb[:U, :], gu.ap())
        w1_sb = const.tile([P, JO, ENC + PRED], f32)  # [j, jo, e|p]
        nc.sync.dma_start(w1_sb[:], w1.ap().rearrange("(jo p) e -> p jo e", p=P))
        w2_sb = const.tile([P, 4, J], f32)            # [v(125 pad 128), vo, j]
        nc.any.memzero(w2_sb[:])
        nc.sync.dma_start(
            w2_sb[:125, :, :], w2.ap().rearrange("(vo p) j -> p vo j", p=125)
        )

        # ---- PE transposes into contraction-major layouts ----
        # Copies alternate DVE/scalar so neither serializes the prologue.
        pst_cm = tc.tile_pool(name="pst", bufs=2, space="PSUM")
        pst = pst_cm.__enter__()

        # ftT[e, t]
        ftT = const.tile([P, T], f32)
        for to in range(TO):
            pt = pst.tile([P, P], f32, tag="pt")
            nc.tensor.transpose(pt[:], ft_sb[:, to, :], ident[:])
            if to == 0:
                nc.vector.tensor_copy(ftT[:, to * P : (to + 1) * P], pt[:])
            else:
                nc.scalar.copy(ftT[:, to * P : (to + 1) * P], pt[:])

        # guT[p, pc, u]
        guT = const.tile([P, PRED // P, U], f32)
        for pc in range(PRED // P):
            pt = pst.tile([P, P], f32, tag="pt")
            nc.tensor.transpose(pt[:], gu_sb[:, pc * P : (pc + 1) * P], ident[:])
            if pc == 0:
                nc.vector.tensor_copy(guT[:, pc, :], pt[:, :U])
            else:
                nc.scalar.copy(guT[:, pc, :], pt[:, :U])

        # w1T[k, kc, j]: w1T[:, kc, jo*P+jj] = w1[jo*P+jj, kc*P+k]
        w1T = const.tile([P, 3, J], f32)
        for kc in range(3):
            for jo in range(JO):
                pt = pst.tile([P, P], f32, tag="pt")
                nc.tensor.transpose(
                    pt[:], w1_sb[:, jo, kc * P : (kc + 1) * P], ident[:]
                )
                if jo == 0:
                    nc.vector.tensor_copy(w1T[:, kc, jo * P : (jo + 1) * P], pt[:])
                else:
                    nc.scalar.copy(w1T[:, kc, jo * P : (jo + 1) * P], pt[:])

        # ---- first GEMMs (fp32): hfT[j, t], hgT[j, u] ----
        hfT = const.tile([P, JO, T], f32)
        for jo in range(JO):
            ph = pst.tile([P, T], f32, tag="ph")
            nc.tensor.matmul(
                ph[:], w1T[:, 0, jo * P : (jo + 1) * P], ftT[:], start=True, stop=True
            )
            if jo == 0:
                nc.vector.tensor_copy(hfT[:, jo, :], ph[:])
            else:
                nc.scalar.copy(hfT[:, jo, :], ph[:])

        hgT = const.tile([P, JO, U], f32)
        for jo in range(JO):
            ph = pst.tile([P, U], f32, tag="phg")
            for pc in range(PRED // P):
                nc.tensor.matmul(
                    ph[:],
                    w1T[:, 1 + pc, jo * P : (jo + 1) * P],
                    guT[:, pc, :],
                    start=(pc == 0),
                    stop=(pc == 1),
                )
            if jo == 0:
                nc.vector.tensor_copy(hgT[:, jo, :], ph[:])
            else:
                nc.scalar.copy(hgT[:, jo, :], ph[:])

        # w2T[j, jo, v] (fp16 for the big GEMM) — transposed last; only
        # needed once the first big matmul issues.
        w2T = const.tile([P, JO, V], f16)
        for vo in range(4):
            for jo in range(JO):
                pt = pst.tile([P, P], f32, tag="pt")
                nc.tensor.transpose(
                    pt[:], w2_sb[:, vo, jo * P : (jo + 1) * P], ident[:]
                )
                if jo == 0:
                    nc.vector.tensor_copy(
                        w2T[:, jo, vo * 125 : (vo + 1) * 125], pt[:, :125]
                    )
                else:
                    nc.scalar.copy(
                        w2T[:, jo, vo * 125 : (vo + 1) * 125], pt[:, :125]
                    )

        pst_cm.__exit__(None, None, None)

        # ---- main loop over u, in blocks of UB for batched output DMA ----
        # Per u: 2 bias'd tanh ACTs (scalar; the bias port makes the
        # hg[u] broadcast free), 4 fp16 matmuls in jo-major order (the
        # first two can issue as soon as the jo=0 tanh lands) into a
        # 2-bank PSUM tile, one fused 1000-elem fp32->fp16 DVE copy out.
        # Scalar ~1.08us/u (tanh) and DVE ~1.06us/u (evac) both sit just
        # at the PE's pace.
        UB = 4
        NBLK = U // UB
        pso_cm = tc.tile_pool(name="pso", bufs=4, space="PSUM")
        pso = pso_cm.__enter__()
        for ub in range(NBLK):
            ot = jpool.tile([P, TO, UB, V], f16, tag="ot", name="ot")
            for uo in range(UB):
                u = ub * UB + uo
                joint = []
                for jo in range(JO):
                    jt = jpool.tile([P, T], f16, tag="joint")
                    nc.scalar.activation(
                        jt[:],
                        hfT[:, jo, :],
                        mybir.ActivationFunctionType.Tanh,
                        bias=hgT[:, jo, u : u + 1],
                        scale=1.0,
                    )
                    joint.append(jt)
                # 512-padded second bank so both matmul outputs are
                # bank-aligned
                po = pso.tile([P, TO, 512], f32, tag="po")
                for to in range(TO):
                    for jo in range(JO):
                        nc.tensor.matmul(
                            po[:, to, 0:V],
                            joint[jo][:, to * P : (to + 1) * P],
                            w2T[:, jo, :],
                            start=(jo == 0),
                            stop=(jo == JO - 1),
                        )
                nc.vector.tensor_copy(ot[:, :, uo, :], po[:, :, 0:V])
            for to in range(TO):
                nc.sync.dma_start(
                    out.ap()[to * P : (to + 1) * P, ub * UB : (ub + 1) * UB, :],
                    ot[:, to, :, :],
                )
        pso_cm.__exit__(None, None, None)


_NC_CACHE = None


def _build():
    global _NC_CACHE
    if _NC_CACHE is not None:
        return _NC_CACHE
    nc = bacc.Bacc("TRN2", target_bir_lowering=False, debug=False)
    ft = nc.dram_tensor("ft", [T, ENC], f32, kind="ExternalInput")
    gu = nc.dram_tensor("gu", [U, PRED], f32, kind="ExternalInput")
    w1 = nc.dram_tensor("w1", [J, ENC + PRED], f32, kind="ExternalInput")
    w2 = nc.dram_tensor("w2", [V, J], f32, kind="ExternalInput")
    out = nc.dram_tensor("out", [T, U, V], f16, kind="ExternalOutput")
    with tile.TileContext(nc) as tc:
        _emit(nc, tc, ft, gu, w1, w2, out)
    nc.compile()
    _NC_CACHE = nc
    return nc


def run(ft, gu, w1, w2, trace=False):
    """Run the SPMD kernel; returns (output [B,T,U,V], BassKernelResults)."""
    nc = _build()
    w1c = np.ascontiguousarray(w1, dtype=np.float32)
    w2c = np.ascontiguousarray(w2, dtype=np.float32)
    in_maps = [
        {
            "ft": np.ascontiguousarray(ft[b], dtype=np.float32),
            "gu": np.ascontiguousarray(gu[b], dtype=np.float32),
            "w1": w1c,
            "w2": w2c,
        }
        for b in range(B)
    ]
    res = run_bass_kernel_spmd(
        nc, in_maps, core_ids=list(range(N_CORES)), trace=trace
    )
    out = np.stack(
        [res.results[c]["out"].astype(np.float32) for c in range(N_CORES)], axis=0
    )
    return out, res


def kernel(ft, gu, w1, w2):
    out, _ = run(ft, gu, w1, w2, trace=False)
    return out


# revision 16
# speedup vs baseline: 4.8093x; 1.0719x over previous
"""RNN-T Joiner kernel for Trainium2, SPMD over 8 NeuronCores.

Reference computation (per batch b):
    hf = ft[b] @ w1[:, :ENC].T            # [T, J]
    hg = gu[b] @ w1[:, ENC:].T            # [U, J]
    joint = tanh(hf[:, None, :] + hg[None, :, :])   # [T, U, J]
    out[b] = joint @ w2.T                 # [T, U, V]

Sharding: data-parallel over B — each of the 8 cores handles one batch
element, full weights replicated. No collectives.

Per-core layout strategy: keep J (the contraction dim of the big GEMM) on
SBUF partitions. hfT[j, t] and hgT[j, u] are produced by small fp32 GEMMs
from PE-transposed inputs. Then for each u, a ScalarE activation computes
jointT[j, t] = tanh(hfT + hgT[:, u]) with the per-partition bias feature,
writing fp16. The big GEMM runs in fp16 (full PE rate, half-cost
LdWeights vs fp32) producing [t-chunk, V] fp32 tiles in PSUM. PSUM
evacuation converts to fp16 and is split across the Vector (DVE) and
GpSimd (Pool) engines so neither throttles the PE. Output is DMA'd to
DRAM in fp16 (halving HBM write traffic vs fp32) and upcast to fp32 on
the host; fp16 quantization error (~1e-3 relative) is far below the
2e-2 gate.
"""

import numpy as np

import concourse.bass as bass
import concourse.mybir as mybir
import concourse.tile as tile
from concourse import bacc
from concourse.bass_utils import run_bass_kernel_spmd
from concourse.masks import make_identity

B, T, U = 8, 256, 64
ENC, PRED = 128, 256
J, V = 256, 500
N_CORES = 8
P = 128
f32 = mybir.dt.float32
f32r = mybir.dt.float32r
f16 = mybir.dt.float16


def _emit(nc, tc, ft, gu, w1, w2, out):
    JO = J // P          # 2 chunks of j
    TO = T // P          # 2 chunks of t
    with (
        tc.tile_pool(name="const", bufs=1) as const,
        tc.tile_pool(name="joint", bufs=8) as jpool,
    ):
        ident = const.tile([P, P], f32)
        make_identity(nc, ident)

        # ---- natural-layout loads ----
        # ft/gu/w1 first: they feed the hfT/hgT critical path; the large
        # w2 load only gates the first big matmul, ~2us later.
        ft_sb = const.tile([P, TO, ENC], f32)         # [t, to, e]
        nc.sync.dma_start(ft_sb[:], ft.ap().rearrange("(to p) e -> p to e", p=P))
        gu_sb = const.tile([P, PRED], f32)            # [u (pad 128), p]
        nc.any.memzero(gu_sb[:])
        nc.sync.dma_start(gu_sb[:U, :], gu.ap())
        w1_sb = const.tile([P, JO, ENC + PRED], f32)  # [j, jo, e|p]
        nc.sync.dma_start(w1_sb[:], w1.ap().rearrange("(jo p) e -> p jo e", p=P))
        w2_sb = const.tile([P, 4, J], f32)            # [v(125 pad 128), vo, j]
        nc.any.memzero(w2_sb[:])
        nc.sync.dma_start(
            w2_sb[:125, :, :], w2.ap().rearrange("(vo p) j -> p vo j", p=125)
        )

        # ---- PE transposes into contraction-major layouts ----
        # Copies alternate DVE/scalar so neither serializes the prologue.
        pst_cm = tc.tile_pool(name="pst", bufs=4, space="PSUM")
        pst = pst_cm.__enter__()
        psg_cm = tc.tile_pool(name="psg", bufs=2, space="PSUM")
        psg = psg_cm.__enter__()

        # ftT[e, t]
        ftT = const.tile([P, T], f32)
        for to in range(TO):
            pt = pst.tile([P, P], f32, tag="pt")
            nc.tensor.transpose(pt[:], ft_sb[:, to, :], ident[:])
            if to == 0:
                nc.vector.tensor_copy(ftT[:, to * P : (to + 1) * P], pt[:])
            else:
                nc.scalar.copy(ftT[:, to * P : (to + 1) * P], pt[:])

        # guT[p, pc, u]
        guT = const.tile([P, PRED // P, U], f32)
        for pc in range(PRED // P):
            pt = pst.tile([P, P], f32, tag="pt")
            nc.tensor.transpose(pt[:], gu_sb[:, pc * P : (pc + 1) * P], ident[:])
            if pc == 0:
                nc.vector.tensor_copy(guT[:, pc, :], pt[:, :U])
            else:
                nc.scalar.copy(guT[:, pc, :], pt[:, :U])

        # w1T[k, kc, j]: w1T[:, kc, jo*P+jj] = w1[jo*P+jj, kc*P+k]
        w1T = const.tile([P, 3, J], f32)
        for kc in range(3):
            for jo in range(JO):
                pt = pst.tile([P, P], f32, tag="pt")
                nc.tensor.transpose(
                    pt[:], w1_sb[:, jo, kc * P : (kc + 1) * P], ident[:]
                )
                if jo == 0:
                    nc.vector.tensor_copy(w1T[:, kc, jo * P : (jo + 1) * P], pt[:])
                else:
                    nc.scalar.copy(w1T[:, kc, jo * P : (jo + 1) * P], pt[:])

        # ---- first GEMMs (fp32): hfT[j, t], hgT[j, u] ----
        hfT = const.tile([P, JO, T], f32)
        for jo in range(JO):
            ph = psg.tile([P, T], f32, tag="ph")
            nc.tensor.matmul(
                ph[:], w1T[:, 0, jo * P : (jo + 1) * P], ftT[:], start=True, stop=True
            )
            if jo == 0:
                nc.vector.tensor_copy(hfT[:, jo, :], ph[:])
            else:
                nc.scalar.copy(hfT[:, jo, :], ph[:])

        hgT = const.tile([P, JO, U], f32)
        for jo in range(JO):
            ph = psg.tile([P, U], f32, tag="phg")
            for pc in range(PRED // P):
                nc.tensor.matmul(
                    ph[:],
                    w1T[:, 1 + pc, jo * P : (jo + 1) * P],
                    guT[:, pc, :],
                    start=(pc == 0),
                    stop=(pc == 1),
                )
            if jo == 0:
                nc.vector.tensor_copy(hgT[:, jo, :], ph[:])
            else:
                nc.scalar.copy(hgT[:, jo, :], ph[:])

        # w2T[j, jo, v] (fp16 for the big GEMM) — transposed last; only
        # needed once the first big matmul issues.
        w2T = const.tile([P, JO, V], f16)
        for vo in range(4):
            for jo in range(JO):
                pt = pst.tile([P, P], f32, tag="pt")
                nc.tensor.transpose(
                    pt[:], w2_sb[:, vo, jo * P : (jo + 1) * P], ident[:]
                )
                nc.vector.tensor_copy(
                    w2T[:, jo, vo * 125 : (vo + 1) * 125], pt[:, :125]
                )

        psg_cm.__exit__(None, None, None)
        pst_cm.__exit__(None, None, None)

        # ---- main loop over u, in blocks of UB for batched output DMA ----
        # Per u: 2 bias'd tanh ACTs (scalar; the bias port makes the
        # hg[u] broadcast free), 4 fp16 matmuls in jo-major order (the
        # first two can issue as soon as the jo=0 tanh lands) into a
        # 2-bank PSUM tile, one fused 1000-elem fp32->fp16 DVE copy out.
        # Scalar ~1.08us/u (tanh) and DVE ~1.06us/u (evac) both sit just
        # at the PE's pace.
        UB = 4
        NBLK = U // UB
        pso_cm = tc.tile_pool(name="pso", bufs=4, space="PSUM")
        pso = pso_cm.__enter__()
        for ub in range(NBLK):
            ot = jpool.tile([P, TO, UB, V], f16, tag="ot", name="ot")
            for uo in range(UB):
                u = ub * UB + uo
                joint = []
                for jo in range(JO):
                    jt = jpool.tile([P, T], f16, tag="joint")
                    nc.scalar.activation(
                        jt[:],
                        hfT[:, jo, :],
                        mybir.ActivationFunctionType.Tanh,
                        bias=hgT[:, jo, u : u + 1],
                        scale=1.0,
                    )
                    joint.append(jt)
                # 512-padded second bank so both matmul outputs are
                # bank-aligned
                po = pso.tile([P, TO, 512], f32, tag="po")
                for to in range(TO):
                    for jo in range(JO):
                        nc.tensor.matmul(
                            po[:, to, 0:V],
                            joint[jo][:, to * P : (to + 1) * P],
                            w2T[:, jo, :],
                            start=(jo == 0),
                            stop=(jo == JO - 1),
                        )
                if u % 21 == 20:
                    nc.scalar.copy(ot[:, :, uo, :], po[:, :, 0:V])
                else:
                    nc.vector.tensor_copy(ot[:, :, uo, :], po[:, :, 0:V])
            for to in range(TO):
                nc.sync.dma_start(
                    out.ap()[to * P : (to + 1) * P, ub * UB : (ub + 1) * UB, :],
                    ot[:, to, :, :],
                )
        pso_cm.__exit__(None, None, None)


_NC_CACHE = None


def _build():
    global _NC_CACHE
    if _NC_CACHE is not None:
        return _NC_CACHE
    nc = bacc.Bacc("TRN2", target_bir_lowering=False, debug=False)
    ft = nc.dram_tensor("ft", [T, ENC], f32, kind="ExternalInput")
    gu = nc.dram_tensor("gu", [U, PRED], f32, kind="ExternalInput")
    w1 = nc.dram_tensor("w1", [J, ENC + PRED], f32, kind="ExternalInput")
    w2 = nc.dram_tensor("w2", [V, J], f32, kind="ExternalInput")
    out = nc.dram_tensor("out", [T, U, V], f16, kind="ExternalOutput")
    with tile.TileContext(nc) as tc:
        _emit(nc, tc, ft, gu, w1, w2, out)
    nc.compile()
    _NC_CACHE = nc
    return nc


def run(ft, gu, w1, w2, trace=False):
    """Run the SPMD kernel; returns (output [B,T,U,V], BassKernelResults)."""
    nc = _build()
    w1c = np.ascontiguousarray(w1, dtype=np.float32)
    w2c = np.ascontiguousarray(w2, dtype=np.float32)
    in_maps = [
        {
            "ft": np.ascontiguousarray(ft[b], dtype=np.float32),
            "gu": np.ascontiguousarray(gu[b], dtype=np.float32),
            "w1": w1c,
            "w2": w2c,
        }
        for b in range(B)
    ]
    res = run_bass_kernel_spmd(
        nc, in_maps, core_ids=list(range(N_CORES)), trace=trace
    )
    out = np.stack(
        [res.results[c]["out"].astype(np.float32) for c in range(N_CORES)], axis=0
    )
    return out, res


def kernel(ft, gu, w1, w2):
    out, _ = run(ft, gu, w1, w2, trace=False)
    return out


# revision 17
# speedup vs baseline: 4.9015x; 1.0192x over previous
"""RNN-T Joiner kernel for Trainium2, SPMD over 8 NeuronCores.

Reference computation (per batch b):
    hf = ft[b] @ w1[:, :ENC].T            # [T, J]
    hg = gu[b] @ w1[:, ENC:].T            # [U, J]
    joint = tanh(hf[:, None, :] + hg[None, :, :])   # [T, U, J]
    out[b] = joint @ w2.T                 # [T, U, V]

Sharding: data-parallel over B — each of the 8 cores handles one batch
element, full weights replicated. No collectives.

Per-core layout strategy: keep J (the contraction dim of the big GEMM) on
SBUF partitions. hfT[j, t] and hgT[j, u] are produced by small fp32 GEMMs
from PE-transposed inputs. Then for each u, a ScalarE activation computes
jointT[j, t] = tanh(hfT + hgT[:, u]) with the per-partition bias feature,
writing fp16. The big GEMM runs in fp16 (full PE rate, half-cost
LdWeights vs fp32) producing [t-chunk, V] fp32 tiles in PSUM. PSUM
evacuation converts to fp16 and is split across the Vector (DVE) and
GpSimd (Pool) engines so neither throttles the PE. Output is DMA'd to
DRAM in fp16 (halving HBM write traffic vs fp32) and upcast to fp32 on
the host; fp16 quantization error (~1e-3 relative) is far below the
2e-2 gate.
"""

import numpy as np

import concourse.bass as bass
import concourse.mybir as mybir
import concourse.tile as tile
from concourse import bacc
from concourse.bass_utils import run_bass_kernel_spmd
from concourse.masks import make_identity

B, T, U = 8, 256, 64
ENC, PRED = 128, 256
J, V = 256, 500
N_CORES = 8
P = 128
f32 = mybir.dt.float32
f32r = mybir.dt.float32r
f16 = mybir.dt.float16


def _emit(nc, tc, ft, gu, w1, w2, out):
    JO = J // P          # 2 chunks of j
    TO = T // P          # 2 chunks of t
    with (
        tc.tile_pool(name="const", bufs=1) as const,
        tc.tile_pool(name="joint", bufs=8) as jpool,
    ):
        ident = const.tile([P, P], f32)
        make_identity(nc, ident)

        # ---- loads: one contiguous multi-KB DRAM run per partition ----
        # Row r lands on partition r//G at sub-index r%G, so each
        # partition line is a single contiguous run (fewer DMA
        # descriptors); the transpose copies below write stride-G to
        # restore true column order. Pad partitions are left
        # uninitialized — the transpose copies discard those columns.
        # Issue from both HWDGE queues (sync + scalar) in parallel.
        ft_sb = const.tile([P, TO, ENC], f32)         # [p, to, e]: t = 2p+to
        nc.sync.dma_start(ft_sb[:], ft.ap().rearrange("(p to) e -> p to e", p=P))
        gu_sb = const.tile([P, PRED], f32)            # [u (64 used), p]
        nc.scalar.dma_start(gu_sb[:U, :], gu.ap())
        w1_sb = const.tile([P, JO, ENC + PRED], f32)  # [p, jo2, e]: j = 2p+jo2
        nc.scalar.dma_start(w1_sb[:], w1.ap().rearrange("(p jo) e -> p jo e", p=P))
        w2_sb = const.tile([P, 4, J], f32)            # [p (125 used), vo, j]: v = 4p+vo
        nc.sync.dma_start(
            w2_sb[:125, :, :], w2.ap().rearrange("(p vo) j -> p vo j", p=125)
        )

        # ---- PE transposes into contraction-major layouts ----
        # Copies alternate DVE/scalar so neither serializes the prologue.
        pst_cm = tc.tile_pool(name="pst", bufs=4, space="PSUM")
        pst = pst_cm.__enter__()
        psg_cm = tc.tile_pool(name="psg", bufs=2, space="PSUM")
        psg = psg_cm.__enter__()

        # ftT[e, i, to]: flat free = t = 2i+to
        ftT = const.tile([P, P, TO], f32)
        for to in range(TO):
            pt = pst.tile([P, P], f32, tag="pt")
            nc.tensor.transpose(pt[:], ft_sb[:, to, :], ident[:])
            if to == 0:
                nc.vector.tensor_copy(ftT[:, :, to], pt[:])
            else:
                nc.scalar.copy(ftT[:, :, to], pt[:])

        # guT[p, pc, u]
        guT = const.tile([P, PRED // P, U], f32)
        for pc in range(PRED // P):
            pt = pst.tile([P, P], f32, tag="pt")
            nc.tensor.transpose(pt[:], gu_sb[:, pc * P : (pc + 1) * P], ident[:])
            if pc == 0:
                nc.vector.tensor_copy(guT[:, pc, :], pt[:, :U])
            else:
                nc.scalar.copy(guT[:, pc, :], pt[:, :U])

        # w1T[k, kc, i, jo2]: flat free = j = 2i+jo2
        w1T = const.tile([P, 3, P, JO], f32)
        for jo2 in range(JO):
            for kc in range(3):
                pt = pst.tile([P, P], f32, tag="pt")
                nc.tensor.transpose(
                    pt[:], w1_sb[:, jo2, kc * P : (kc + 1) * P], ident[:]
                )
                if jo2 == 0:
                    nc.vector.tensor_copy(w1T[:, kc, :, jo2], pt[:])
                else:
                    nc.scalar.copy(w1T[:, kc, :, jo2], pt[:])

        # ---- first GEMMs (fp32): hfT[j, t], hgT[j, u] ----
        hfT = const.tile([P, JO, T], f32)
        for jo in range(JO):
            ph = psg.tile([P, T], f32, tag="ph")
            nc.tensor.matmul(
                ph[:],
                w1T[:, 0, 64 * jo : 64 * (jo + 1), :],
                ftT[:],
                start=True,
                stop=True,
            )
            if jo == 0:
                nc.vector.tensor_copy(hfT[:, jo, :], ph[:])
            else:
                nc.scalar.copy(hfT[:, jo, :], ph[:])

        hgT = const.tile([P, JO, U], f32)
        for jo in range(JO):
            ph = psg.tile([P, U], f32, tag="phg")
            for pc in range(PRED // P):
                nc.tensor.matmul(
                    ph[:],
                    w1T[:, 1 + pc, 64 * jo : 64 * (jo + 1), :],
                    guT[:, pc, :],
                    start=(pc == 0),
                    stop=(pc == 1),
                )
            if jo == 0:
                nc.vector.tensor_copy(hgT[:, jo, :], ph[:])
            else:
                nc.scalar.copy(hgT[:, jo, :], ph[:])

        # w2T[j, jo, i, vo]: flat free = v = 4i+vo (fp16 for the big
        # GEMM) — transposed last; only needed once the first big matmul
        # issues.
        w2T = const.tile([P, JO, 125, 4], f16)
        for vo in range(4):
            for jo in range(JO):
                pt = pst.tile([P, P], f32, tag="pt")
                nc.tensor.transpose(
                    pt[:], w2_sb[:, vo, jo * P : (jo + 1) * P], ident[:]
                )
                nc.vector.tensor_copy(w2T[:, jo, :, vo], pt[:, :125])

        psg_cm.__exit__(None, None, None)
        pst_cm.__exit__(None, None, None)

        # ---- main loop over u, in blocks of UB for batched output DMA ----
        # Per u: 2 bias'd tanh ACTs (scalar; the bias port makes the
        # hg[u] broadcast free), 4 fp16 matmuls in jo-major order (the
        # first two can issue as soon as the jo=0 tanh lands) into a
        # 2-bank PSUM tile, one fused 1000-elem fp32->fp16 DVE copy out.
        # Scalar ~1.08us/u (tanh) and DVE ~1.06us/u (evac) both sit just
        # at the PE's pace.
        UB = 4
        NBLK = U // UB
        pso_cm = tc.tile_pool(name="pso", bufs=4, space="PSUM")
        pso = pso_cm.__enter__()
        for ub in range(NBLK):
            ot = jpool.tile([P, TO, UB, V], f16, tag="ot", name="ot")
            for uo in range(UB):
                u = ub * UB + uo
                joint = []
                for jo in range(JO):
                    jt = jpool.tile([P, T], f16, tag="joint")
                    nc.scalar.activation(
                        jt[:],
                        hfT[:, jo, :],
                        mybir.ActivationFunctionType.Tanh,
                        bias=hgT[:, jo, u : u + 1],
                        scale=1.0,
                    )
                    joint.append(jt)
                # 512-padded second bank so both matmul outputs are
                # bank-aligned
                po = pso.tile([P, TO, 512], f32, tag="po")
                for to in range(TO):
                    for jo in range(JO):
                        nc.tensor.matmul(
                            po[:, to, 0:V],
                            joint[jo][:, to * P : (to + 1) * P],
                            w2T[:, jo, :, :],
                            start=(jo == 0),
                            stop=(jo == JO - 1),
                        )
                if u % 21 == 20:
                    nc.scalar.copy(ot[:, :, uo, :], po[:, :, 0:V])
                else:
                    nc.vector.tensor_copy(ot[:, :, uo, :], po[:, :, 0:V])
            for to in range(TO):
                nc.sync.dma_start(
                    out.ap()[to * P : (to + 1) * P, ub * UB : (ub + 1) * UB, :],
                    ot[:, to, :, :],
                )
        pso_cm.__exit__(None, None, None)


_NC_CACHE = None


def _build():
    global _NC_CACHE
    if _NC_CACHE is not None:
        return _NC_CACHE
    nc = bacc.Bacc("TRN2", target_bir_lowering=False, debug=False)
    ft = nc.dram_tensor("ft", [T, ENC], f32, kind="ExternalInput")
    gu = nc.dram_tensor("gu", [U, PRED], f32, kind="ExternalInput")
    w1 = nc.dram_tensor("w1", [J, ENC + PRED], f32, kind="ExternalInput")
    w2 = nc.dram_tensor("w2", [V, J], f32, kind="ExternalInput")
    out = nc.dram_tensor("out", [T, U, V], f16, kind="ExternalOutput")
    with tile.TileContext(nc) as tc:
        _emit(nc, tc, ft, gu, w1, w2, out)
    nc.compile()
    _NC_CACHE = nc
    return nc


def run(ft, gu, w1, w2, trace=False):
    """Run the SPMD kernel; returns (output [B,T,U,V], BassKernelResults)."""
    nc = _build()
    w1c = np.ascontiguousarray(w1, dtype=np.float32)
    w2c = np.ascontiguousarray(w2, dtype=np.float32)
    in_maps = [
        {
            "ft": np.ascontiguousarray(ft[b], dtype=np.float32),
            "gu": np.ascontiguousarray(gu[b], dtype=np.float32),
            "w1": w1c,
            "w2": w2c,
        }
        for b in range(B)
    ]
    res = run_bass_kernel_spmd(
        nc, in_maps, core_ids=list(range(N_CORES)), trace=trace
    )
    out = np.stack(
        [res.results[c]["out"].astype(np.float32) for c in range(N_CORES)], axis=0
    )
    return out, res


def kernel(ft, gu, w1, w2):
    out, _ = run(ft, gu, w1, w2, trace=False)
    return out
